# revision 9
# baseline (speedup 1.0000x reference)
"""Trainium2 Bass kernel: bilinear edge decoder (GNN message passing).

scores[e] = z[edge_index[0, e]] @ W @ z[edge_index[1, e]]

Strategy (8 NeuronCores, edge-parallel, no cross-core comm):
  - Shard the 600k edges across 8 cores (75k each); replicate z and W.
  - z is passed as packed bf16 hi|lo rows ([N, 256] bf16: hi = bf16(z),
    lo = bf16(z - hi); hi+lo carries ~16 mantissa bits). Rows stay 512 B, so
    the gather traffic is unchanged while enabling fast bf16 matmuls.
  - The gather uses the SWDGE `dma_gather` ucode path, 4 SWDGE queues deep
    (queue depth hides the per-descriptor HBM read latency). Indices are
    int16, so the host buckets each core's edges by (src>>15, dst>>15) into
    16 buckets; a bucket's calls read from the matching 32768-row slice of
    z. Buckets are padded to 128-edge tiles and to a common per-bucket tile
    count across cores (SPMD: one program for all 8 cores).
  - Per 4-tile batch: one batched SBUF->SBUF DMA transpose (xbar) produces
    hiT|loT [d, e] blocks; per tile 3 accumulating bf16 matmuls compute
    src @ W to fp32-pair accuracy (hi@Whi + hi@Wlo + lo@Whi, fp32 PSUM);
    DVE computes dsum = dhi+dlo (f32), multiplies with the PSUM result and
    reduces -> 4 score columns.

kernel(**inputs) takes FULL unsharded inputs, returns the FULL [600000] f32
score vector.
"""

import hashlib
import os

import ml_dtypes
import numpy as np

import concourse.bass as bass
import concourse.tile as tile
import concourse.mybir as mybir
from concourse import bacc
import concourse.bass2jax as b2j
from concourse.bass_utils import run_bass_kernel_spmd

P = 128
N_NODES = 100000
FEAT = 128
N_CORES = 8
CHUNK = 32768
N_CHUNKS = -(-N_NODES // CHUNK)  # 4
G = 32  # tiles per gather call (4096 rows = 2 MB)

# ---------------------------------------------------------------------------
# Compile cache + NEFF capture: walrus compiles are minutes; cache the
# compiled custom-call bytes keyed on the HLO so repeat runs are instant.
_NEFF_CACHE = os.environ.get("BASS_NEFF_CACHE", "/tmp/bass_neff_cache")
LAST_NEFF = os.path.join(_NEFF_CACHE, "last.neff")

if not getattr(b2j, "_ant_cache_patched", False):
    b2j._ant_cache_patched = True
    _orig_hook = b2j.neuronx_cc_hook
    _orig_rename = b2j.rename_neff_tensors_and_patch_header

    def _rename_and_save(neff_file, renames):
        data = _orig_rename(neff_file, renames)
        try:
            os.makedirs(_NEFF_CACHE, exist_ok=True)
            with open(LAST_NEFF + ".tmp", "wb") as f:
                f.write(data)
            os.replace(LAST_NEFF + ".tmp", LAST_NEFF)
        except OSError:
            pass
        return data

    def _cached_hook(code, code_format, platform_version, file_prefix):
        key = hashlib.sha256(bytes(code)).hexdigest()
        path = os.path.join(_NEFF_CACHE, key + ".bin")
        neff_copy = os.path.join(_NEFF_CACHE, key + ".neff")
        if os.path.exists(path):
            if os.path.exists(neff_copy):
                try:
                    import shutil

                    shutil.copy(neff_copy, LAST_NEFF)
                except OSError:
                    pass
            with open(path, "rb") as f:
                return 0, f.read()
        r = _orig_hook(code, code_format, platform_version, file_prefix)
        try:
            os.makedirs(_NEFF_CACHE, exist_ok=True)
            with open(path + ".tmp", "wb") as f:
                f.write(r[1])
            os.replace(path + ".tmp", path)
            if os.path.exists(LAST_NEFF):
                import shutil

                shutil.copy(LAST_NEFF, neff_copy)
        except OSError:
            pass
        return r

    b2j.rename_neff_tensors_and_patch_header = _rename_and_save
    b2j.neuronx_cc_hook = _cached_hook

# Tile assigns DMASW completion-sem lanes round-robin in scheduled order,
# which can disagree with explicit dma_gather queue_num (each lane's sem is
# locked to one SWDGE queue by the ucode). Pin lane = queue_num.
import concourse.tile_sem_assignment as _tsa
from concourse.tile_scheduler import DMAInst as _DMAInst

if not getattr(_tsa, "_ant_queue_lane_patched", False):
    _tsa._ant_queue_lane_patched = True
    _orig_assign_tick = _tsa.TileClockTick._assign_tick

    def _assign_tick_q(self, inst):
        qn = getattr(inst, "queue_num", None)
        if (
            qn is not None
            and isinstance(inst, _DMAInst)
            and inst.engine == mybir.EngineType.Pool
        ):
            self.next_sw_dma_idx = qn % self.swdge_sem_count
        return _orig_assign_tick(self, inst)

    _tsa.TileClockTick._assign_tick = _assign_tick_q


# ---------------------------------------------------------------------------
def _build_nc(schedule, t_sched, num_devices):
    """schedule: list of (src_chunk, dst_chunk, n_tiles)."""
    nc = bacc.Bacc(
        "TRN2",
        target_bir_lowering=False,
        debug=False,
        enable_asserts=False,
        num_devices=num_devices,
        num_swdge_queues=4,
    )
    f32 = mybir.dt.float32
    bf = mybir.dt.bfloat16
    i16 = mybir.dt.int16
    zp = nc.dram_tensor("zp", [N_NODES, 2 * FEAT], bf, kind="ExternalInput").ap()
    whi = nc.dram_tensor("whi", [FEAT, FEAT], bf, kind="ExternalInput").ap()
    wlo = nc.dram_tensor("wlo", [FEAT, FEAT], bf, kind="ExternalInput").ap()
    si = nc.dram_tensor("si", [P, t_sched * 8], i16, kind="ExternalInput").ap()
    di = nc.dram_tensor("di", [P, t_sched * 8], i16, kind="ExternalInput").ap()
    out = nc.dram_tensor("scores", [P, t_sched], f32, kind="ExternalOutput").ap()

    with tile.TileContext(nc) as tc:
        with (
            tc.tile_pool(name="const", bufs=1) as const_pool,
            tc.tile_pool(name="stage", bufs=2) as stage_pool,
            tc.tile_pool(name="work", bufs=4) as work_pool,
            tc.tile_pool(name="psB", bufs=3, space="PSUM") as psB,
        ):
            whi_sb = const_pool.tile([P, FEAT], bf)
            nc.sync.dma_start(whi_sb[:], whi)
            wlo_sb = const_pool.tile([P, FEAT], bf)
            nc.sync.dma_start(wlo_sb[:], wlo)
            si_sb = const_pool.tile([P, t_sched * 8], i16)
            nc.sync.dma_start(si_sb[:], si)
            di_sb = const_pool.tile([P, t_sched * 8], i16)
            nc.sync.dma_start(di_sb[:], di)
            scores_sb = const_pool.tile([P, t_sched], f32)

            t0 = 0
            qctr = 0
            for a, c, n_b in schedule:
                za = zp[a * CHUNK : min((a + 1) * CHUNK, N_NODES), :]
                zc = zp[c * CHUNK : min((c + 1) * CHUNK, N_NODES), :]
                done = 0
                while done < n_b:
                    gt = min(G, n_b - done)
                    kc = gt * P
                    src_stage = stage_pool.tile([P, G * 2 * P], bf, tag="src_stage")
                    dst_stage = stage_pool.tile([P, G * 2 * P], bf, tag="dst_stage")
                    col0 = t0 * 8
                    nc.gpsimd.dma_gather(
                        src_stage[:, : kc * 2].rearrange("p (b f) -> p b f", f=2 * P),
                        za,
                        si_sb[:, col0 : col0 + kc // 16],
                        kc,
                        kc,
                        2 * P,
                        single_packet=False,
                        queue_num=qctr % 4,
                    )
                    qctr += 1
                    nc.gpsimd.dma_gather(
                        dst_stage[:, : kc * 2].rearrange("p (b f) -> p b f", f=2 * P),
                        zc,
                        di_sb[:, col0 : col0 + kc // 16],
                        kc,
                        kc,
                        2 * P,
                        single_packet=False,
                        queue_num=qctr % 4,
                    )
                    qctr += 1
                    jj = 0
                    while jj < gt:
                        bt = min(4, gt - jj)
                        tp = work_pool.tile([P, 4 * 2 * P], bf, tag="tp")
                        nc.sync.dma_start(
                            tp[:, : bt * 2 * P].rearrange("p (k f) -> p k f", f=P),
                            src_stage[:, jj * 2 * P : (jj + bt) * 2 * P],
                            transpose=True,
                        )
                        y_ps = psB.tile([P, 4 * P], f32, tag="y_ps")
                        for u in range(bt):
                            hiT = tp[:, (2 * u) * P : (2 * u + 1) * P]
                            loT = tp[:, (2 * u + 1) * P : (2 * u + 2) * P]
                            yq = y_ps[:, u * P : (u + 1) * P]
                            nc.tensor.matmul(
                                yq, lhsT=hiT, rhs=whi_sb[:], start=True, stop=False
                            )
                            nc.tensor.matmul(
                                yq, lhsT=hiT, rhs=wlo_sb[:], start=False, stop=False
                            )
                            nc.tensor.matmul(
                                yq, lhsT=loT, rhs=whi_sb[:], start=False, stop=True
                            )
                        dpack = dst_stage[
                            :, jj * 2 * P : (jj + bt) * 2 * P
                        ].rearrange("p (b two f) -> p b two f", two=2, f=P)
                        dsum = work_pool.tile([P, 4 * P], f32, tag="dsum")
                        nc.vector.tensor_tensor(
                            out=dsum[:, : bt * P].rearrange("p (b f) -> p b f", f=P),
                            in0=dpack[:, :, 0, :],
                            in1=dpack[:, :, 1, :],
                            op=mybir.AluOpType.add,
                        )
                        prod = work_pool.tile([P, 4 * P], f32, tag="prod")
                        nc.vector.tensor_tensor(
                            out=prod[:, : bt * P],
                            in0=y_ps[:, : bt * P],
                            in1=dsum[:, : bt * P],
                            op=mybir.AluOpType.mult,
                        )
                        nc.vector.tensor_reduce(
                            out=scores_sb[:, t0 + jj : t0 + jj + bt],
                            in_=prod[:, : bt * P].rearrange("p (a b) -> p a b", b=P),
                            axis=mybir.AxisListType.X,
                            op=mybir.AluOpType.add,
                        )
                        jj += bt
                    done += gt
                    t0 += gt
            nc.sync.dma_start(out, scores_sb[:])
    nc.compile()
    return nc


# ---------------------------------------------------------------------------
def _wrap_idx16(rel):
    """[K] int16 -> wrapped [128, K/16]: idx i at [i%16, i//16], tiled x8."""
    base = rel.reshape(-1, 16).T  # [16, K/16]
    return np.tile(base, (8, 1))


def pack_z(z):
    z = np.asarray(z, dtype=np.float32)
    hi = z.astype(ml_dtypes.bfloat16)
    lo = (z - hi.astype(np.float32)).astype(ml_dtypes.bfloat16)
    zp = np.empty((z.shape[0], 2 * z.shape[1]), ml_dtypes.bfloat16)
    zp[:, : z.shape[1]] = hi
    zp[:, z.shape[1] :] = lo
    return np.ascontiguousarray(zp)


def split_w(W):
    W = np.asarray(W, dtype=np.float32)
    hi = W.astype(ml_dtypes.bfloat16)
    lo = (W - hi.astype(np.float32)).astype(ml_dtypes.bfloat16)
    return np.ascontiguousarray(hi), np.ascontiguousarray(lo)


def plan(edge_index):
    """Host planning: shard, bucket, pad. Returns everything kernel() needs."""
    ei = np.asarray(edge_index)
    n_edges = ei.shape[1]
    per_core = -(-n_edges // N_CORES)

    src_all = ei[0].astype(np.int64)
    dst_all = ei[1].astype(np.int64)

    core_buckets = []  # [core][bucket] -> local edge ids
    for c in range(N_CORES):
        lo, hi = c * per_core, min((c + 1) * per_core, n_edges)
        s = src_all[lo:hi]
        d = dst_all[lo:hi]
        b = (s // CHUNK) * N_CHUNKS + (d // CHUNK)
        buckets = []
        order = np.argsort(b, kind="stable")
        bs = b[order]
        bounds = np.searchsorted(bs, np.arange(N_CHUNKS * N_CHUNKS + 1))
        for bi in range(N_CHUNKS * N_CHUNKS):
            buckets.append(order[bounds[bi] : bounds[bi + 1]])
        core_buckets.append(buckets)

    # shared schedule: per-bucket tile count = max over cores
    schedule = []
    for bi in range(N_CHUNKS * N_CHUNKS):
        n_max = max(len(core_buckets[c][bi]) for c in range(N_CORES))
        n_tiles = -(-n_max // P)
        if n_tiles:
            schedule.append((bi // N_CHUNKS, bi % N_CHUNKS, n_tiles))
    t_sched = sum(n for _, _, n in schedule)

    # per-core index arrays + permutation
    si_list, di_list, perm_list = [], [], []
    for c in range(N_CORES):
        lo, hi = c * per_core, min((c + 1) * per_core, n_edges)
        s = src_all[lo:hi]
        d = dst_all[lo:hi]
        si_cols, di_cols, perms = [], [], []
        for a, cck, n_tiles in schedule:
            ids = core_buckets[c][a * N_CHUNKS + cck]
            nslots = n_tiles * P
            rel_s = np.zeros(nslots, np.int16)
            rel_d = np.zeros(nslots, np.int16)
            pm = np.full(nslots, -1, np.int64)
            k = len(ids)
            rel_s[:k] = (s[ids] - a * CHUNK).astype(np.int16)
            rel_d[:k] = (d[ids] - cck * CHUNK).astype(np.int16)
            pm[:k] = ids
            done = 0
            while done < n_tiles:
                gt = min(G, n_tiles - done)
                kc = gt * P
                seg = slice(done * P, done * P + kc)
                si_cols.append(_wrap_idx16(rel_s[seg]))
                di_cols.append(_wrap_idx16(rel_d[seg]))
                done += gt
            perms.append(pm)
        si_list.append(np.ascontiguousarray(np.concatenate(si_cols, axis=1)))
        di_list.append(np.ascontiguousarray(np.concatenate(di_cols, axis=1)))
        perm_list.append(np.concatenate(perms))

    return schedule, t_sched, si_list, di_list, perm_list, per_core, n_edges


_nc_cache = {}


def kernel(z, edge_index, W):
    zp = pack_z(z)
    whi, wlo = split_w(W)
    schedule, t_sched, si_list, di_list, perm_list, per_core, n_edges = plan(
        edge_index
    )

    key = (tuple(schedule), t_sched)
    if key not in _nc_cache:
        _nc_cache[key] = _build_nc(schedule, t_sched, N_CORES)
    nc = _nc_cache[key]

    in_maps = [
        {"zp": zp, "whi": whi, "wlo": wlo, "si": si_list[c], "di": di_list[c]}
        for c in range(N_CORES)
    ]
    res = run_bass_kernel_spmd(nc, in_maps, core_ids=list(range(N_CORES)))

    out = np.empty(n_edges, np.float32)
    for c in range(N_CORES):
        lo, hi = c * per_core, min((c + 1) * per_core, n_edges)
        scores_lin = res.results[c]["scores"].T.reshape(-1)  # k = t*128+p
        pm = perm_list[c]
        mask = pm >= 0
        out[lo + pm[mask]] = scores_lin[mask]
    return out


if __name__ == "__main__":
    rng = np.random.default_rng(0)
    z = rng.standard_normal((N_NODES, FEAT)).astype(np.float32)
    ei = rng.integers(0, N_NODES, size=(2, 600000)).astype(np.int64)
    W = (rng.standard_normal((FEAT, FEAT)) * 0.09).astype(np.float32)
    s = kernel(z, ei, W)
    exp = np.einsum("ed,df,ef->e", z[ei[0]], W, z[ei[1]], optimize=True)
    rel = np.abs(s - exp).max() / np.abs(exp).max()
    print("rel err:", rel)
